# revision 21
# baseline (speedup 1.0000x reference)
"""AdmissibleStatesHead on 8 Trainium2 NeuronCores.

marginals[c] = segment_sum(softmax(E @ W.T + b), digit_c)  ==  P @ M_c
where M is a one-hot [N_VALID, 48] matrix built on host from valid_states.

Device work per core (valid-states sharded 8 ways, batch replicated):
  logits^T tile [128v, 512b] = sum_k wt[k,v].T @ et[k,b]   (fp8 DoubleRow PE, fp32 PSUM)
  exp tile = Exp(logits^T * inv_scale + bias)              (ScalarE, PSUM -> SBUF fp8)
  U^T [48, 512b] += M_chunk.T @ exp_tile                   (fp8 DoubleRow PE)
Host: sum per-core partials, normalize by concept-0 bucket sum (= softmax
denominator), reshape to [6, B, 8]. W is pre-scaled by a power of two into
fp8's range; the Exp activation's free affine undoes it.

Set KERNEL_BF16=1 for a bf16 fallback (~1.8x slower, ~15x more accurate);
KERNEL_TRACE=1 captures an NTFF profile and fills LAST_EXEC_NS.
"""

import os
import sys
import types

import numpy as np
import ml_dtypes

OUTCOMES = [8, 8, 8, 8, 8, 8]
N_TOTAL = 262144
N_VALID = 8192
B, D = 4096, 1024
N_CORES = 8
P = 128
V_S = N_VALID // N_CORES  # 1024 valid states per core
NK = D // P               # 8 contraction chunks
NV = V_S // P             # 8 v-tiles per core
NB = B // 512             # 8 batch tiles of 512
NJ = 48                   # 6 concepts x 8 outcomes

# W values are small (~N(0, 0.02^2) per spec); scale into fp8e4m3's normal
# range and undo the scale for free inside the Exp activation. Chosen per
# call from the data as a power of two; the compiled module is cached per
# scale value.
DEFAULT_W_SCALE = 64.0

USE_BF16 = bool(os.environ.get("KERNEL_BF16"))

LAST_EXEC_NS = None
LAST_RESULT = None
_compiled_cache = {}


def _pick_w_scale(wmax):
    import math

    if not np.isfinite(wmax) or wmax <= 0:
        return DEFAULT_W_SCALE
    # keep max|W*scale| around <=192 (fp8e4m3 max 448), scale a power of 2
    s = 2.0 ** math.floor(math.log2(192.0 / wmax))
    return float(min(max(s, 2.0 ** -10), 2.0 ** 20))


def _split_excess_waits(nc, limit=1):
    """This walrus build rejects instructions carrying more than ~1 sync-wait
    ("Too many sync wait commands"). Hoist excess waits onto injected NoOps
    right before the instruction on the same engine — sequencers are in-order,
    so the semantics are identical."""
    import concourse.mybir as mybir

    ctr = 0
    main_bb = nc.m.functions[0].blocks[0]
    stripped = []
    for ins in main_bb.instructions:
        nm = str(ins.name)
        op = ins.concise_opcode()
        if op == "Drain" or (op == "EventSemaphore" and nm.startswith("barrier_")):
            continue
        stripped.append(ins)
    main_bb.instructions = stripped
    for fn in nc.m.functions:
        for bb in fn.blocks:
            insts = bb.instructions
            new = []
            changed = False
            for ins in insts:
                si = ins.sync_info
                lim = 1 if ins.concise_opcode() == "Drain" else limit
                if si is not None and len(si.on_wait) > lim:
                    waits = list(si.on_wait)
                    for w in waits[:-lim]:
                        ctr += 1
                        nop = mybir.InstNoOp(name=f"waitsplit_{ctr}", ins=[], outs=[])
                        nop.engine = ins.engine
                        nop.sync_info = mybir.SyncInfo(on_update=[], on_wait=[w])
                        new.append(nop)
                    ins.sync_info = mybir.SyncInfo(
                        on_update=list(si.on_update), on_wait=waits[-lim:]
                    )
                    changed = True
                new.append(ins)
            if changed:
                bb.instructions = new


def _patch_tile_tail():
    import concourse.tile as tile
    from concourse.vector_clock import ScopedClock

    if getattr(tile.TileContext, "_tail_patched", False):
        return

    def _drain_and_barrier(self, tick_clock, wait_clock):
        drain_inst = self.nc.sync.drain()
        wait_clock.add_sem_waits(
            drain_inst.ins, ScopedClock({None: tick_clock.global_clock})
        )
        self.nc.all_engine_barrier()
        popped = self.nc._tile_sem_poison_stack.pop()
        assert popped is self._sem_poison
        self.nc.clear_and_free_semaphores(list(self.sems.allocated().values()))

    tile.TileContext._drain_and_barrier = _drain_and_barrier
    tile.TileContext._tail_patched = True


def _build_nc(w_scale):
    import concourse.bass as bass
    import concourse.mybir as mybir
    import concourse.tile as tile

    _patch_tile_tail()

    f32 = mybir.dt.float32
    bf16 = mybir.dt.bfloat16
    fp8 = mybir.dt.float8e4
    Exp = mybir.ActivationFunctionType.Exp

    in_dt = fp8
    exp_scale = 1.0 / w_scale
    n_warm = int(os.environ.get("KERNEL_WARM", "7"))

    nc = bass.Bass()
    # h interleaves [wt k-pair block (2KB) | et tile-0 k-pair block (1KB)]
    # per partition, so each k-pair group of batch tile 0 is gated by one
    # large-line transfer; k-pair-major weight order also serves tiles 1-7.
    h = nc.dram_tensor("h", [P, NK // 2, 3072], in_dt, kind="ExternalInput")
    et = nc.dram_tensor("et", [P, NB - 1, NK, 512], in_dt, kind="ExternalInput")
    mm = nc.dram_tensor("mm", [P, NV, NJ], in_dt, kind="ExternalInput")
    bias = nc.dram_tensor("bias", [P, NV], f32, kind="ExternalInput")
    # slot NB holds the second half-partial of the last batch tile (its
    # segment matmul is split in two to shorten the tail chain); the host
    # adds slots NB-1 and NB.
    out = nc.dram_tensor("out", [NJ, NB + 1, 512], bf16, kind="ExternalOutput")

    with (
        tile.TileContext(nc) as tc,
        tc.tile_pool(name="const", bufs=1) as cpool,
        tc.tile_pool(name="expp", bufs=3) as xpool,
        tc.tile_pool(name="ps", bufs=6, space="PSUM") as pspool,
        tc.tile_pool(name="ps2", bufs=2, space="PSUM") as ps2pool,
        tc.tile_pool(name="uo", bufs=2) as upool,
        tc.tile_pool(name="warm", bufs=1) as wpool,
    ):
        # All input triggers ride the Activation HWDGE ring in priority
        # order (the ring is FIFO at packet granularity across the 16 SDMA
        # engines). Output stores go on the otherwise-idle SP ring: an
        # HWDGE trigger waits at the *issuing sequencer*, which would stall
        # the exp pipeline if the stores were issued from Activation.
        h_sb = cpool.tile([P, NK // 2, 3072], in_dt)
        et_sb = cpool.tile([P, NB - 1, NK, 512], in_dt)
        b_sb = cpool.tile([P, NV], f32)
        m_sb = cpool.tile([P, NV, NJ], in_dt)
        nc.scalar.dma_start(h_sb[:, 0], h[:, 0])
        nc.scalar.dma_start(b_sb[:], bias[:])
        nc.scalar.dma_start(h_sb[:, 1], h[:, 1])
        nc.scalar.dma_start(h_sb[:, 2], h[:, 2])
        nc.scalar.dma_start(h_sb[:, 3], h[:, 3])
        nc.scalar.dma_start(et_sb[:, 0:1], et[:, 0:1])
        nc.scalar.dma_start(m_sb[:], mm[:])
        nc.scalar.dma_start(et_sb[:, 1:3], et[:, 1:3])
        nc.scalar.dma_start(et_sb[:, 3:7], et[:, 3:7])

        wt_kp = [
            h_sb[:, kp, 0:2048].rearrange("p (ko v q) -> p ko v q", ko=2, v=NV, q=P)
            for kp in range(NK // 2)
        ]
        et0_kp = [
            h_sb[:, kp, 2048:3072].rearrange("p (ko n) -> p ko n", ko=2, n=512)
            for kp in range(NK // 2)
        ]

        # PE HAM warm-up: the clock gate only opens after ~3.4us of
        # sustained PE activity; bridge the input-DMA window with throwaway
        # matmuls so batch tile 0 streams at 2.4 GHz.
        warm_sb = wpool.tile([P, 512], in_dt)
        nc.gpsimd.memset(warm_sb[:], 0)
        warm_act = wpool.tile([P, 16], in_dt)
        # pull the ~2.7us Exp table load into the DMA-wait window
        nc.scalar.activation(warm_act[:], warm_sb[:, 0:16], Exp)
        # borrow a slot from the main psum pool; it returns to the
        # rotation once the warm-up matmuls are done
        warm_ps = pspool.tile([P, 512], f32, tag="ps")
        for _ in range(n_warm - 1):
            nc.tensor.matmul(
                warm_ps[:],
                lhsT=warm_sb[:, 0:P],
                rhs=warm_sb[:],
                start=True,
                stop=True,
            )
        # finer-grained warm-up tail: the first real matmul queues behind
        # the last warm-up, so quantize the end of the warm-up window in
        # ~110ns steps instead of 430ns
        for _ in range(2):
            nc.tensor.matmul(
                warm_ps[:, 0:256],
                lhsT=warm_sb[:, 0:P],
                rhs=warm_sb[:, 0:256],
                start=True,
                stop=True,
            )

        def emit_mm2_mms(exp_t, vlo, vhi, ups):
            for v in range(vlo, vhi, 2):
                nc.tensor.matmul(
                    ups[:],
                    lhsT=m_sb[:, v : v + 2, :],
                    rhs=exp_t[:, v : v + 2, :],
                    start=(v == vlo),
                    stop=(v == vhi - 2),
                    perf_mode=mybir.MatmulPerfMode.DoubleRow,
                )

        def emit_mm2(n, exp_t):
            ups = ps2pool.tile([NJ, 512], f32, tag="ups")
            emit_mm2_mms(exp_t, 0, NV, ups)
            u_sb = upool.tile([NJ, 512], bf16, tag="u")
            nc.vector.tensor_copy(u_sb[:], ups[:])
            nc.sync.dma_start(out[:, n, :], u_sb[:])

        # Batch tile 0 runs k-pair-outer across all 8 PSUM banks: its first
        # matmuls need only the first h transfer, so real work starts while
        # the rest of the inputs stream in.
        exp0 = xpool.tile([P, NV, 512], in_dt, tag="exp")
        ps0 = [
            pspool.tile([P, 512], f32, tag="ps", name=f"ps0_{i}")
            for i in range(6)
        ] + [
            ps2pool.tile([P, 512], f32, tag="ups", name=f"ps0_{6 + i}")
            for i in range(2)
        ]
        for kp in range(NK // 2):
            rhs = et0_kp[kp]
            for v in range(NV):
                nc.tensor.matmul(
                    ps0[v][:],
                    lhsT=wt_kp[kp][:, :, v, :],
                    rhs=rhs,
                    start=(kp == 0),
                    stop=(kp == NK // 2 - 1),
                    perf_mode=mybir.MatmulPerfMode.DoubleRow,
                )
        for v in range(NV):
            nc.scalar.activation(
                exp0[:, v, :], ps0[v][:], Exp, bias=b_sb[:, v : v + 1],
                scale=exp_scale,
            )

        # MM2 of tile n is deferred to tile n+1's v==6 slot: tile 0's exps
        # drain through ScalarE in a ~5.5us burst, so an earlier slot would
        # park the PE queue on an exp-complete wait.
        pending = (0, exp0)
        for n in range(1, NB):
            exp_t = xpool.tile([P, NV, 512], in_dt, tag="exp")
            last = n == NB - 1
            # the last batch tile runs as two 256-wide halves so the
            # exec-end chain (exp + segment matmul + cast + store) is half
            # as long; its first half is flushed while the second computes
            for half in range(2 if last else 1):
                cols = slice(half * 256, half * 256 + 256) if last else slice(0, 512)
                for v in range(NV):
                    ps = pspool.tile([P, 512], f32, tag="ps")
                    for k in range(0, NK, 2):
                        nc.tensor.matmul(
                            ps[:, cols],
                            lhsT=wt_kp[k // 2][:, :, v, :],
                            rhs=et_sb[:, n - 1, k : k + 2, cols],
                            start=(k == 0),
                            stop=(k == NK - 2),
                            perf_mode=mybir.MatmulPerfMode.DoubleRow,
                        )
                    nc.scalar.activation(
                        exp_t[:, v, cols], ps[:, cols], Exp,
                        bias=b_sb[:, v : v + 1], scale=exp_scale,
                    )
                    if pending is not None and half == 0 and v == 6:
                        emit_mm2(*pending)
                        pending = None
                if last:
                    ups_h = ps2pool.tile([NJ, 512], f32, tag="ups")
                    for v in range(0, NV, 2):
                        nc.tensor.matmul(
                            ups_h[:, cols],
                            lhsT=m_sb[:, v : v + 2, :],
                            rhs=exp_t[:, v : v + 2, cols],
                            start=(v == 0),
                            stop=(v == NV - 2),
                            perf_mode=mybir.MatmulPerfMode.DoubleRow,
                        )
                    u_h = upool.tile([NJ, 512], bf16, tag="u")
                    nc.vector.tensor_copy(u_h[:, cols], ups_h[:, cols])
                    nc.sync.dma_start(out[:, NB - 1 + half, cols], u_h[:, cols])
            if not last:
                pending = (n, exp_t)
    _split_excess_waits(nc)
    return nc


def _install_ntff_hook():
    """bass_utils' axon trace path imports antenv.axon_hooks, absent in this
    image; shim it using trn_boot's ctypes NTFF hook."""
    if "antenv.axon_hooks" in sys.modules:
        return
    try:
        from trn_agent_boot.trn_boot import _ntff_profile_via_ctypes

        hook = _ntff_profile_via_ctypes("/opt/axon/libaxon_pjrt.so")
    except Exception:
        hook = None
    mod = types.ModuleType("antenv.axon_hooks")
    mod.get_axon_ntff_profile_hook = lambda: hook
    sys.modules["antenv.axon_hooks"] = mod


def kernel(embeddings, W, b, valid_states):
    global LAST_EXEC_NS, LAST_RESULT
    assert not USE_BF16, "bf16 fallback removed"
    E = np.asarray(embeddings, dtype=np.float32)
    Wf = np.asarray(W, dtype=np.float32)
    bf = np.asarray(b, dtype=np.float32)
    vs = np.asarray(valid_states).astype(np.int64)

    in_dt = ml_dtypes.float8_e4m3
    w_scale = _pick_w_scale(float(np.abs(Wf).max()))
    Wp = Wf * w_scale

    # etk[k, p, n, j] = E[n*512+j, k*128+p]
    etk = E.T.astype(in_dt).reshape(NK, P, NB, 512)
    # tiles 1..NB-1, 32KB-contiguous per partition
    et_host = np.ascontiguousarray(etk[:, :, 1:, :].transpose(1, 2, 0, 3))
    # et tile 0's k-pair blocks, interleaved into the head tensor below
    et0_blk = np.ascontiguousarray(
        etk[:, :, 0, :].reshape(NK // 2, 2, P, 512).transpose(2, 0, 1, 3)
    ).reshape(P, NK // 2, 1024)

    # One-hot segment matrix M [N_VALID, 48]
    M = np.zeros((N_VALID, NJ), dtype=in_dt)
    stride = N_TOTAL
    for c, n_i in enumerate(OUTCOMES):
        stride //= n_i
        digit = (vs // stride) % n_i
        M[np.arange(N_VALID), c * 8 + digit] = 1

    in_maps = []
    for core in range(N_CORES):
        sl = slice(core * V_S, (core + 1) * V_S)
        # wk[k, p, v, q] = W[v*128+q, k*128+p] * scale
        wk = Wp[sl, :].T.astype(in_dt).reshape(NK, P, NV, P)
        h_host = np.empty((P, NK // 2, 3072), dtype=in_dt)
        h_host[:, :, 0:2048] = (
            wk.reshape(NK // 2, 2, P, NV, P).transpose(2, 0, 1, 3, 4)
            .reshape(P, NK // 2, 2048)
        )
        h_host[:, :, 2048:3072] = et0_blk
        m_host = np.ascontiguousarray(M[sl].reshape(NV, P, NJ).transpose(1, 0, 2))
        b_host = np.ascontiguousarray(bf[sl].reshape(NV, P).T)
        in_maps.append(
            {"h": h_host, "et": et_host, "mm": m_host, "bias": b_host}
        )

    from concourse.bass_utils import run_bass_kernel_spmd

    key = w_scale
    if key not in _compiled_cache:
        _compiled_cache[key] = _build_nc(w_scale)
    nc_mod = _compiled_cache[key]

    kwargs = {}
    if os.environ.get("KERNEL_TRACE"):
        _install_ntff_hook()
        kwargs["trace"] = True

    res = run_bass_kernel_spmd(
        nc_mod, in_maps, core_ids=list(range(N_CORES)), **kwargs
    )
    LAST_EXEC_NS = res.exec_time_ns
    LAST_RESULT = res

    U = np.zeros((NJ, B), dtype=np.float64)
    for r in res.results:
        o = r["out"].astype(np.float64)  # [NJ, NB+1, 512]
        # last tile is stored as two 256-wide halves in slots NB-1 and NB
        o[:, NB - 1, 256:] = o[:, NB, 256:]
        U += o[:, :NB].reshape(NJ, B)
    denom = U[0:8].sum(axis=0)  # [B] total softmax denominator
    marg = U.reshape(6, 8, B) / denom  # [6, 8, B]
    return np.ascontiguousarray(marg.transpose(0, 2, 1)).astype(np.float32)


# revision 22
# speedup vs baseline: 1.0034x; 1.0034x over previous
"""AdmissibleStatesHead on 8 Trainium2 NeuronCores.

marginals[c] = segment_sum(softmax(E @ W.T + b), digit_c)  ==  P @ M_c
where M is a one-hot [N_VALID, 48] matrix built on host from valid_states.

Device work per core (valid-states sharded 8 ways, batch replicated):
  logits^T tile [128v, 512b] = sum_k wt[k,v].T @ et[k,b]   (fp8 DoubleRow PE, fp32 PSUM)
  exp tile = Exp(logits^T * inv_scale + bias)              (ScalarE, PSUM -> SBUF fp8)
  U^T [48, 512b] += M_chunk.T @ exp_tile                   (fp8 DoubleRow PE)
Host: sum per-core partials, normalize by concept-0 bucket sum (= softmax
denominator), reshape to [6, B, 8]. W is pre-scaled by a power of two into
fp8's range; the Exp activation's free affine undoes it.

Schedule notes (PE is the bottleneck at ~62us of matmul; everything else
is latency engineering):
  - all input DMA triggers ride the Activation HWDGE ring in priority
    order; a "head" tensor interleaves [wt k-pair | et tile-0 k-pair]
    blocks so batch tile 0 can start on one 384KB transfer
  - batch tile 0 runs k-pair-outer across all 8 PSUM banks while later
    inputs stream in; remaining tiles run v-outer
  - a few throwaway matmuls bridge the HAM clock-gate warm-up (~3.4us)
    during the input-DMA window
  - tile n's segment matmul is deferred into tile n+1 (v==6) to dodge the
    ScalarE exp backlog; the last tile runs as two 256-wide halves so the
    exec-end chain is short
Output is stored as bf16 partials ([NJ, NB+1, 512]; the last two slots
are the half-tiles), summed and normalized on host in float64.

KERNEL_TRACE=1 captures an NTFF profile and fills LAST_EXEC_NS;
KERNEL_WARM overrides the warm-up matmul count (default 7).
"""

import os
import sys
import types

import numpy as np
import ml_dtypes

OUTCOMES = [8, 8, 8, 8, 8, 8]
N_TOTAL = 262144
N_VALID = 8192
B, D = 4096, 1024
N_CORES = 8
P = 128
V_S = N_VALID // N_CORES  # 1024 valid states per core
NK = D // P               # 8 contraction chunks
NV = V_S // P             # 8 v-tiles per core
NB = B // 512             # 8 batch tiles of 512
NJ = 48                   # 6 concepts x 8 outcomes

# W values are small (~N(0, 0.02^2) per spec); scale into fp8e4m3's normal
# range and undo the scale for free inside the Exp activation. Chosen per
# call from the data as a power of two; the compiled module is cached per
# scale value.
DEFAULT_W_SCALE = 64.0

USE_BF16 = bool(os.environ.get("KERNEL_BF16"))

LAST_EXEC_NS = None
LAST_RESULT = None
_compiled_cache = {}


def _pick_w_scale(wmax):
    import math

    if not np.isfinite(wmax) or wmax <= 0:
        return DEFAULT_W_SCALE
    # keep max|W*scale| around <=192 (fp8e4m3 max 448), scale a power of 2
    s = 2.0 ** math.floor(math.log2(192.0 / wmax))
    return float(min(max(s, 2.0 ** -10), 2.0 ** 20))


def _split_excess_waits(nc, limit=1):
    """This walrus build rejects instructions carrying more than ~1 sync-wait
    ("Too many sync wait commands"). Hoist excess waits onto injected NoOps
    right before the instruction on the same engine — sequencers are in-order,
    so the semantics are identical."""
    import concourse.mybir as mybir

    ctr = 0
    main_bb = nc.m.functions[0].blocks[0]
    stripped = []
    for ins in main_bb.instructions:
        nm = str(ins.name)
        op = ins.concise_opcode()
        if op == "Drain" or (op == "EventSemaphore" and nm.startswith("barrier_")):
            continue
        stripped.append(ins)
    main_bb.instructions = stripped
    for fn in nc.m.functions:
        for bb in fn.blocks:
            insts = bb.instructions
            new = []
            changed = False
            for ins in insts:
                si = ins.sync_info
                lim = 1 if ins.concise_opcode() == "Drain" else limit
                if si is not None and len(si.on_wait) > lim:
                    waits = list(si.on_wait)
                    for w in waits[:-lim]:
                        ctr += 1
                        nop = mybir.InstNoOp(name=f"waitsplit_{ctr}", ins=[], outs=[])
                        nop.engine = ins.engine
                        nop.sync_info = mybir.SyncInfo(on_update=[], on_wait=[w])
                        new.append(nop)
                    ins.sync_info = mybir.SyncInfo(
                        on_update=list(si.on_update), on_wait=waits[-lim:]
                    )
                    changed = True
                new.append(ins)
            if changed:
                bb.instructions = new


def _patch_tile_tail():
    import concourse.tile as tile
    from concourse.vector_clock import ScopedClock

    if getattr(tile.TileContext, "_tail_patched", False):
        return

    def _drain_and_barrier(self, tick_clock, wait_clock):
        drain_inst = self.nc.sync.drain()
        wait_clock.add_sem_waits(
            drain_inst.ins, ScopedClock({None: tick_clock.global_clock})
        )
        self.nc.all_engine_barrier()
        popped = self.nc._tile_sem_poison_stack.pop()
        assert popped is self._sem_poison
        self.nc.clear_and_free_semaphores(list(self.sems.allocated().values()))

    tile.TileContext._drain_and_barrier = _drain_and_barrier
    tile.TileContext._tail_patched = True


def _build_nc(w_scale):
    import concourse.bass as bass
    import concourse.mybir as mybir
    import concourse.tile as tile

    _patch_tile_tail()

    f32 = mybir.dt.float32
    bf16 = mybir.dt.bfloat16
    fp8 = mybir.dt.float8e4
    Exp = mybir.ActivationFunctionType.Exp

    in_dt = fp8
    exp_scale = 1.0 / w_scale
    n_warm = int(os.environ.get("KERNEL_WARM", "7"))

    nc = bass.Bass()
    # h interleaves [wt k-pair block (2KB) | et tile-0 k-pair block (1KB)]
    # per partition, so each k-pair group of batch tile 0 is gated by one
    # large-line transfer; k-pair-major weight order also serves tiles 1-7.
    h = nc.dram_tensor("h", [P, NK // 2, 3072], in_dt, kind="ExternalInput")
    et = nc.dram_tensor("et", [P, NB - 1, NK, 512], in_dt, kind="ExternalInput")
    mm = nc.dram_tensor("mm", [P, NV, NJ], in_dt, kind="ExternalInput")
    bias = nc.dram_tensor("bias", [P, NV], f32, kind="ExternalInput")
    # slot NB holds the second half-partial of the last batch tile (its
    # segment matmul is split in two to shorten the tail chain); the host
    # adds slots NB-1 and NB.
    out = nc.dram_tensor("out", [NJ, NB + 1, 512], bf16, kind="ExternalOutput")

    with (
        tile.TileContext(nc) as tc,
        tc.tile_pool(name="const", bufs=1) as cpool,
        tc.tile_pool(name="expp", bufs=3) as xpool,
        tc.tile_pool(name="ps", bufs=6, space="PSUM") as pspool,
        tc.tile_pool(name="ps2", bufs=2, space="PSUM") as ps2pool,
        tc.tile_pool(name="uo", bufs=2) as upool,
        tc.tile_pool(name="warm", bufs=1) as wpool,
    ):
        # All input triggers ride the Activation HWDGE ring in priority
        # order (the ring is FIFO at packet granularity across the 16 SDMA
        # engines). Output stores go on the otherwise-idle SP ring: an
        # HWDGE trigger waits at the *issuing sequencer*, which would stall
        # the exp pipeline if the stores were issued from Activation.
        h_sb = cpool.tile([P, NK // 2, 3072], in_dt)
        et_sb = cpool.tile([P, NB - 1, NK, 512], in_dt)
        b_sb = cpool.tile([P, NV], f32)
        m_sb = cpool.tile([P, NV, NJ], in_dt)
        nc.scalar.dma_start(h_sb[:, 0], h[:, 0])
        nc.scalar.dma_start(b_sb[:], bias[:])
        nc.scalar.dma_start(h_sb[:, 1], h[:, 1])
        nc.scalar.dma_start(h_sb[:, 2], h[:, 2])
        nc.scalar.dma_start(h_sb[:, 3], h[:, 3])
        nc.scalar.dma_start(et_sb[:, 0:1], et[:, 0:1])
        nc.scalar.dma_start(m_sb[:], mm[:])
        nc.scalar.dma_start(et_sb[:, 1:3], et[:, 1:3])
        nc.scalar.dma_start(et_sb[:, 3:7], et[:, 3:7])

        wt_kp = [
            h_sb[:, kp, 0:2048].rearrange("p (ko v q) -> p ko v q", ko=2, v=NV, q=P)
            for kp in range(NK // 2)
        ]
        et0_kp = [
            h_sb[:, kp, 2048:3072].rearrange("p (ko n) -> p ko n", ko=2, n=512)
            for kp in range(NK // 2)
        ]

        # PE HAM warm-up: the clock gate only opens after ~3.4us of
        # sustained PE activity; bridge the input-DMA window with throwaway
        # matmuls so batch tile 0 streams at 2.4 GHz.
        warm_sb = wpool.tile([P, 512], in_dt)
        nc.gpsimd.memset(warm_sb[:], 0)
        warm_act = wpool.tile([P, 16], in_dt)
        # pull the ~2.7us Exp table load into the DMA-wait window
        nc.scalar.activation(warm_act[:], warm_sb[:, 0:16], Exp)
        # borrow a slot from the main psum pool; it returns to the
        # rotation once the warm-up matmuls are done
        warm_ps = pspool.tile([P, 512], f32, tag="ps")
        for _ in range(n_warm - 1):
            nc.tensor.matmul(
                warm_ps[:],
                lhsT=warm_sb[:, 0:P],
                rhs=warm_sb[:],
                start=True,
                stop=True,
            )
        # finer-grained warm-up tail: the first real matmul queues behind
        # the last warm-up, so quantize the end of the warm-up window in
        # ~110ns steps instead of 430ns
        for _ in range(2):
            nc.tensor.matmul(
                warm_ps[:, 0:256],
                lhsT=warm_sb[:, 0:P],
                rhs=warm_sb[:, 0:256],
                start=True,
                stop=True,
            )

        def emit_mm2_mms(exp_t, vlo, vhi, ups):
            for v in range(vlo, vhi, 2):
                nc.tensor.matmul(
                    ups[:],
                    lhsT=m_sb[:, v : v + 2, :],
                    rhs=exp_t[:, v : v + 2, :],
                    start=(v == vlo),
                    stop=(v == vhi - 2),
                    perf_mode=mybir.MatmulPerfMode.DoubleRow,
                )

        def emit_mm2(n, exp_t):
            ups = ps2pool.tile([NJ, 512], f32, tag="ups")
            emit_mm2_mms(exp_t, 0, NV, ups)
            u_sb = upool.tile([NJ, 512], bf16, tag="u")
            nc.vector.tensor_copy(u_sb[:], ups[:])
            nc.sync.dma_start(out[:, n, :], u_sb[:])

        # Batch tile 0 runs k-pair-outer across all 8 PSUM banks: its first
        # matmuls need only the first h transfer, so real work starts while
        # the rest of the inputs stream in.
        exp0 = xpool.tile([P, NV, 512], in_dt, tag="exp")
        ps0 = [
            pspool.tile([P, 512], f32, tag="ps", name=f"ps0_{i}")
            for i in range(6)
        ] + [
            ps2pool.tile([P, 512], f32, tag="ups", name=f"ps0_{6 + i}")
            for i in range(2)
        ]
        for kp in range(NK // 2):
            rhs = et0_kp[kp]
            for v in range(NV):
                nc.tensor.matmul(
                    ps0[v][:],
                    lhsT=wt_kp[kp][:, :, v, :],
                    rhs=rhs,
                    start=(kp == 0),
                    stop=(kp == NK // 2 - 1),
                    perf_mode=mybir.MatmulPerfMode.DoubleRow,
                )
        for v in range(NV):
            nc.scalar.activation(
                exp0[:, v, :], ps0[v][:], Exp, bias=b_sb[:, v : v + 1],
                scale=exp_scale,
            )

        # MM2 of tile n is deferred to tile n+1's v==6 slot: tile 0's exps
        # drain through ScalarE in a ~5.5us burst, so an earlier slot would
        # park the PE queue on an exp-complete wait.
        pending = (0, exp0)
        for n in range(1, NB):
            exp_t = xpool.tile([P, NV, 512], in_dt, tag="exp")
            last = n == NB - 1
            # the last batch tile runs as two 256-wide halves so the
            # exec-end chain (exp + segment matmul + cast + store) is half
            # as long; its first half is flushed while the second computes
            for half in range(2 if last else 1):
                cols = slice(half * 256, half * 256 + 256) if last else slice(0, 512)
                for v in range(NV):
                    ps = pspool.tile([P, 512], f32, tag="ps")
                    for k in range(0, NK, 2):
                        nc.tensor.matmul(
                            ps[:, cols],
                            lhsT=wt_kp[k // 2][:, :, v, :],
                            rhs=et_sb[:, n - 1, k : k + 2, cols],
                            start=(k == 0),
                            stop=(k == NK - 2),
                            perf_mode=mybir.MatmulPerfMode.DoubleRow,
                        )
                    nc.scalar.activation(
                        exp_t[:, v, cols], ps[:, cols], Exp,
                        bias=b_sb[:, v : v + 1], scale=exp_scale,
                    )
                    if pending is not None and half == 0 and v == 6:
                        emit_mm2(*pending)
                        pending = None
                if last:
                    ups_h = ps2pool.tile([NJ, 512], f32, tag="ups")
                    for v in range(0, NV, 2):
                        nc.tensor.matmul(
                            ups_h[:, cols],
                            lhsT=m_sb[:, v : v + 2, :],
                            rhs=exp_t[:, v : v + 2, cols],
                            start=(v == 0),
                            stop=(v == NV - 2),
                            perf_mode=mybir.MatmulPerfMode.DoubleRow,
                        )
                    u_h = upool.tile([NJ, 512], bf16, tag="u")
                    nc.vector.tensor_copy(u_h[:, cols], ups_h[:, cols])
                    nc.sync.dma_start(out[:, NB - 1 + half, cols], u_h[:, cols])
            if not last:
                pending = (n, exp_t)
    _split_excess_waits(nc)
    return nc


def _install_ntff_hook():
    """bass_utils' axon trace path imports antenv.axon_hooks, absent in this
    image; shim it using trn_boot's ctypes NTFF hook."""
    if "antenv.axon_hooks" in sys.modules:
        return
    try:
        from trn_agent_boot.trn_boot import _ntff_profile_via_ctypes

        hook = _ntff_profile_via_ctypes("/opt/axon/libaxon_pjrt.so")
    except Exception:
        hook = None
    mod = types.ModuleType("antenv.axon_hooks")
    mod.get_axon_ntff_profile_hook = lambda: hook
    sys.modules["antenv.axon_hooks"] = mod


def kernel(embeddings, W, b, valid_states):
    global LAST_EXEC_NS, LAST_RESULT
    assert not USE_BF16, "bf16 fallback removed"
    E = np.asarray(embeddings, dtype=np.float32)
    Wf = np.asarray(W, dtype=np.float32)
    bf = np.asarray(b, dtype=np.float32)
    vs = np.asarray(valid_states).astype(np.int64)

    in_dt = ml_dtypes.float8_e4m3
    w_scale = _pick_w_scale(float(np.abs(Wf).max()))
    Wp = Wf * w_scale

    # etk[k, p, n, j] = E[n*512+j, k*128+p]
    etk = E.T.astype(in_dt).reshape(NK, P, NB, 512)
    # tiles 1..NB-1, 32KB-contiguous per partition
    et_host = np.ascontiguousarray(etk[:, :, 1:, :].transpose(1, 2, 0, 3))
    # et tile 0's k-pair blocks, interleaved into the head tensor below
    et0_blk = np.ascontiguousarray(
        etk[:, :, 0, :].reshape(NK // 2, 2, P, 512).transpose(2, 0, 1, 3)
    ).reshape(P, NK // 2, 1024)

    # One-hot segment matrix M [N_VALID, 48]
    M = np.zeros((N_VALID, NJ), dtype=in_dt)
    stride = N_TOTAL
    for c, n_i in enumerate(OUTCOMES):
        stride //= n_i
        digit = (vs // stride) % n_i
        M[np.arange(N_VALID), c * 8 + digit] = 1

    in_maps = []
    for core in range(N_CORES):
        sl = slice(core * V_S, (core + 1) * V_S)
        # wk[k, p, v, q] = W[v*128+q, k*128+p] * scale
        wk = Wp[sl, :].T.astype(in_dt).reshape(NK, P, NV, P)
        h_host = np.empty((P, NK // 2, 3072), dtype=in_dt)
        h_host[:, :, 0:2048] = (
            wk.reshape(NK // 2, 2, P, NV, P).transpose(2, 0, 1, 3, 4)
            .reshape(P, NK // 2, 2048)
        )
        h_host[:, :, 2048:3072] = et0_blk
        m_host = np.ascontiguousarray(M[sl].reshape(NV, P, NJ).transpose(1, 0, 2))
        b_host = np.ascontiguousarray(bf[sl].reshape(NV, P).T)
        in_maps.append(
            {"h": h_host, "et": et_host, "mm": m_host, "bias": b_host}
        )

    from concourse.bass_utils import run_bass_kernel_spmd

    key = w_scale
    if key not in _compiled_cache:
        _compiled_cache[key] = _build_nc(w_scale)
    nc_mod = _compiled_cache[key]

    kwargs = {}
    if os.environ.get("KERNEL_TRACE"):
        _install_ntff_hook()
        kwargs["trace"] = True

    res = run_bass_kernel_spmd(
        nc_mod, in_maps, core_ids=list(range(N_CORES)), **kwargs
    )
    LAST_EXEC_NS = res.exec_time_ns
    LAST_RESULT = res

    U = np.zeros((NJ, B), dtype=np.float64)
    for r in res.results:
        o = r["out"].astype(np.float64)  # [NJ, NB+1, 512]
        # last tile is stored as two 256-wide halves in slots NB-1 and NB
        o[:, NB - 1, 256:] = o[:, NB, 256:]
        U += o[:, :NB].reshape(NJ, B)
    denom = U[0:8].sum(axis=0)  # [B] total softmax denominator
    marg = U.reshape(6, 8, B) / denom  # [6, 8, B]
    return np.ascontiguousarray(marg.transpose(0, 2, 1)).astype(np.float32)
